# revision 6
# baseline (speedup 1.0000x reference)
"""MAAC critic kernel for Trainium2, data-parallel over the batch dim on 8 cores.

Self-contained: hardcodes all shapes. The harness calls kernel(**inputs) with the
full (unsharded) inputs and gets the full [A, B, 1] output back.
"""
import os
import numpy as np
import ml_dtypes

import concourse.bass as bass
import concourse.tile as tile
import concourse.mybir as mybir
from concourse import bacc
from concourse import bass_utils

F32 = mybir.dt.float32
BF16 = mybir.dt.bfloat16
bfloat16 = ml_dtypes.bfloat16

A = 8
B = 16384
S = 456
NACT = 16
IDIM = S + NACT          # 472
KPAD = 512               # padded encoder contraction dim
H = 256
E = 4
D = 64
N_CORES = 8
B_LOCAL = B // N_CORES   # 2048
NB = 256                 # chunk size along b
ALPHA = 0.01             # leaky relu slope

# attention work split: agents < GP_SPLIT run on VectorE, the rest on GPSIMD
GP_SPLIT = 5


def _ap(base, dims):
    """AP with base's partition dim + explicit free [step, count] dims (elements)."""
    return bass.AP(tensor=base.tensor, offset=base.offset, ap=[base.ap[0], *dims])


def build_bass(b_local=B_LOCAL, lrelu_act=True):
    """Build and compile the single-core Bass module (same NEFF on all cores)."""
    nc = bacc.Bacc("TRN2", target_bir_lowering=False, debug=False)
    n_chunks = b_local // NB
    LRELU = mybir.ActivationFunctionType.Lrelu
    RELU = mybir.ActivationFunctionType.Relu
    EXP = mybir.ActivationFunctionType.Exp
    IDENT = mybir.ActivationFunctionType.Identity
    MULT = mybir.AluOpType.mult
    ADD = mybir.AluOpType.add
    ISGE = mybir.AluOpType.is_ge
    X = mybir.AxisListType.X

    # ---- DRAM I/O ----
    saT_d = nc.dram_tensor("saT", [A, KPAD, b_local], BF16, kind="ExternalInput")
    acts_d = nc.dram_tensor("acts", [b_local, A, NACT], F32, kind="ExternalInput")
    w_enc_d = nc.dram_tensor("w_enc", [A, KPAD, 2 * H], BF16, kind="ExternalInput")
    b_enc_d = nc.dram_tensor("b_enc", [A, 4, 128, 1], F32, kind="ExternalInput")
    w_kvs_d = nc.dram_tensor("w_kvs", [H, 3 * H], BF16, kind="ExternalInput")
    b_val_d = nc.dram_tensor("b_val", [2, 128, 1], F32, kind="ExternalInput")
    w_c1_d = nc.dram_tensor("w_c1", [A, 2 * H, H], BF16, kind="ExternalInput")
    b_c1_d = nc.dram_tensor("b_c1", [A, 2, 128, 1], F32, kind="ExternalInput")
    w_c2_d = nc.dram_tensor("w_c2", [A, H, NACT], BF16, kind="ExternalInput")
    b_c2_d = nc.dram_tensor("b_c2", [A, NACT, 1], F32, kind="ExternalInput")
    mask_d = nc.dram_tensor("mask", [128, A * E * A], BF16, kind="ExternalInput")
    q_d = nc.dram_tensor("q", [A, b_local], F32, kind="ExternalOutput")

    with tile.TileContext(nc) as tc:
        import contextlib
        with contextlib.ExitStack() as ctx:
            wp = ctx.enter_context(tc.tile_pool(name="wp", bufs=1))
            xp = ctx.enter_context(tc.tile_pool(name="xp", bufs=3))
            ep = ctx.enter_context(tc.tile_pool(name="ep", bufs=3))      # per-agent fm tiles
            sp = ctx.enter_context(tc.tile_pool(name="sp", bufs=2))      # s_enc (chunk-lived)
            bmp = ctx.enter_context(tc.tile_pool(name="bmp", bufs=2))    # b-major kvs
            pp = ctx.enter_context(tc.tile_pool(name="pp", bufs=3))      # product scratch
            ap_ = ctx.enter_context(tc.tile_pool(name="ap", bufs=2))     # attention small
            op_ = ctx.enter_context(tc.tile_pool(name="op", bufs=2))     # other / gather
            cp = ctx.enter_context(tc.tile_pool(name="cp", bufs=3))      # critic per-agent
            psp = ctx.enter_context(tc.tile_pool(name="psp", bufs=3, space="PSUM"))
            qsp = ctx.enter_context(tc.tile_pool(name="qsp", bufs=2, space="PSUM"))
            tpp = ctx.enter_context(tc.tile_pool(name="tpp", bufs=2, space="PSUM"))

            # ---- resident weights ----
            w_enc_sb = []
            b_enc_sb = []
            b_enc2_sb = []
            w_c1_sb = []
            b_c1_sb = []
            b_c12_sb = []
            w_c2_sb = []
            b_c2_sb = []
            for a in range(A):
                w = wp.tile([128, 4, 2 * H], BF16, name=f"w_enc{a}")
                nc.sync.dma_start(out=w, in_=w_enc_d[a].rearrange("(kt p) m -> p kt m", p=128))
                w_enc_sb.append(w)
                bt_ = wp.tile([128, 4, 1], F32, name=f"b_enc{a}")
                nc.sync.dma_start(out=bt_, in_=b_enc_d[a].rearrange("kt p one -> p kt one"))
                b_enc_sb.append(bt_)
                w1 = wp.tile([128, 4, H], BF16, name=f"w_c1{a}")
                nc.sync.dma_start(out=w1, in_=w_c1_d[a].rearrange("(kt p) m -> p kt m", p=128))
                w_c1_sb.append(w1)
                b1 = wp.tile([128, 2, 1], F32, name=f"b_c1{a}")
                nc.sync.dma_start(out=b1, in_=b_c1_d[a].rearrange("kt p one -> p kt one"))
                b_c1_sb.append(b1)
                w2 = wp.tile([128, 2, NACT], BF16, name=f"w_c2{a}")
                nc.sync.dma_start(out=w2, in_=w_c2_d[a].rearrange("(kt p) m -> p kt m", p=128))
                w_c2_sb.append(w2)
                b2 = wp.tile([NACT, 1], F32, name=f"b_c2{a}")
                nc.sync.dma_start(out=b2, in_=b_c2_d[a])
                b_c2_sb.append(b2)
                if not lrelu_act:
                    bt2 = wp.tile([128, 4, 1], F32, name=f"b_enc2{a}")
                    nc.vector.tensor_scalar_mul(out=bt2, in0=bt_, scalar1=-(1.0 - ALPHA))
                    b_enc2_sb.append(bt2)
                    b12 = wp.tile([128, 2, 1], F32, name=f"b_c12{a}")
                    nc.vector.tensor_scalar_mul(out=b12, in0=b1, scalar1=-(1.0 - ALPHA))
                    b_c12_sb.append(b12)
            w_kvs_sb = wp.tile([128, 2, 3 * H], BF16, name="w_kvs")
            nc.sync.dma_start(out=w_kvs_sb, in_=w_kvs_d.rearrange("(kt p) m -> p kt m", p=128))
            b_val_sb = wp.tile([128, 2, 1], F32, name="b_val")
            nc.sync.dma_start(out=b_val_sb, in_=b_val_d.rearrange("kt p one -> p kt one"))
            b_val2_sb = None
            if not lrelu_act:
                b_val2_sb = wp.tile([128, 2, 1], F32, name="b_val2")
                nc.vector.tensor_scalar_mul(out=b_val2_sb, in0=b_val_sb, scalar1=-(1.0 - ALPHA))
            mask_sb = wp.tile([128, A * E * A], BF16, name="mask")
            nc.sync.dma_start(out=mask_sb, in_=mask_d[:, :])
            ident16 = wp.tile([16, 16], F32, name="ident16")
            from concourse.masks import make_identity
            make_identity(nc, ident16)

            def evict_lrelu(psum, out_ap, bias, bias2):
                """out = lrelu(psum + bias), PSUM -> SBUF."""
                if lrelu_act:
                    nc.scalar.activation(out=out_ap, in_=psum, func=LRELU,
                                         bias=bias, scale=1.0, alpha=ALPHA)
                else:
                    r = ep.tile([128, NB], F32, tag="lrtmp")
                    nc.scalar.activation(out=r, in_=psum, func=RELU,
                                         bias=bias2, scale=-(1.0 - ALPHA))
                    nc.vector.scalar_tensor_tensor(out=out_ap, in0=psum, scalar=bias,
                                                   op0=ADD, in1=r, op1=ADD)

            for c in range(n_chunks):
                c0 = c * NB
                nbt = NB // 128  # b-tiles per chunk (2)

                s_enc_all = sp.tile([128, A, 2, NB], BF16, tag="s_enc")
                keys_bm = [bmp.tile([128, A, E * D], BF16, tag=f"keys{bt}", name=f"keys{bt}") for bt in range(nbt)]
                vals_bm = [bmp.tile([128, A, E * D], BF16, tag=f"vals{bt}", name=f"vals{bt}") for bt in range(nbt)]
                sels_bm = [bmp.tile([128, A, E * D], BF16, tag=f"sels{bt}", name=f"sels{bt}") for bt in range(nbt)]

                # ---------- feature-major phase (per agent) ----------
                for a in range(A):
                    x = xp.tile([128, 4, NB], BF16, tag="x")
                    nc.sync.dma_start(
                        out=x,
                        in_=saT_d[a].rearrange("(kt p) b -> p kt b", p=128)[:, :, c0:c0 + NB])

                    sa_enc = ep.tile([128, 2, NB], BF16, tag="sa_enc")
                    for mt in range(4):
                        ps = psp.tile([128, NB], F32, tag="mm")
                        for kt in range(4):
                            nc.tensor.matmul(out=ps,
                                             lhsT=w_enc_sb[a][:, kt, mt * 128:(mt + 1) * 128],
                                             rhs=x[:, kt, :],
                                             start=(kt == 0), stop=(kt == 3))
                        dst = sa_enc[:, mt, :] if mt < 2 else s_enc_all[:, a, mt - 2, :]
                        evict_lrelu(ps, dst, b_enc_sb[a][:, mt, :],
                                    b_enc2_sb[a][:, mt, :] if not lrelu_act else None)

                    kvs_fm = ep.tile([128, 6, NB], BF16, tag="kvs_fm")
                    for m6 in range(6):
                        ps = psp.tile([128, NB], F32, tag="mm")
                        for kt in range(2):
                            rhs = sa_enc[:, kt, :] if m6 < 4 else s_enc_all[:, a, kt, :]
                            nc.tensor.matmul(out=ps,
                                             lhsT=w_kvs_sb[:, kt, m6 * 128:(m6 + 1) * 128],
                                             rhs=rhs,
                                             start=(kt == 0), stop=(kt == 1))
                        if 2 <= m6 < 4:  # vals: bias + lrelu
                            evict_lrelu(ps, kvs_fm[:, m6, :], b_val_sb[:, m6 - 2, :],
                                        b_val2_sb[:, m6 - 2, :] if not lrelu_act else None)
                        else:            # keys / sels: plain copy-cast
                            nc.vector.tensor_copy(out=kvs_fm[:, m6, :], in_=ps)

                    # transpose to b-major ([128f, 128b] -> [128b, 128f] chunks)
                    for m6 in range(6):
                        dst_t = (keys_bm, vals_bm, sels_bm)[m6 // 2]
                        ht = m6 % 2
                        for bt in range(nbt):
                            nc.sync.dma_start_transpose(
                                out=dst_t[bt][:, a, ht * 128:(ht + 1) * 128],
                                in_=kvs_fm[:, m6, bt * 128:(bt + 1) * 128])

                # ---------- attention phase (per b-tile of 128) ----------
                other_bm = [op_.tile([128, A, E * D], BF16, tag=f"other{bt}", name=f"other{bt}") for bt in range(nbt)]
                for bt in range(nbt):
                    l_t = ap_.tile([128, A, E, A], F32, tag="l")      # (i, e, j)
                    for i in range(A):
                        eng = nc.vector if i < GP_SPLIT else nc.gpsimd
                        prod = pp.tile([128, A, E, D], BF16, tag="prod")  # (j, e, d)
                        # sels_bm: [128, i, (e,d)] ; keys_bm: [128, j, (e,d)]
                        s_base = sels_bm[bt][:, i, 0:1]
                        k_base = keys_bm[bt][:, 0, 0:1]
                        eng.tensor_tensor(
                            out=_ap(prod[:, 0, 0, 0:1], [[256, A], [64, E], [1, D]]),
                            in0=_ap(s_base, [[0, A], [64, E], [1, D]]),
                            in1=_ap(k_base, [[256, A], [64, E], [1, D]]),
                            op=MULT)
                        # d-tree: sum over d (innermost), (j,e) kept
                        width = D
                        while width > 2:
                            h2 = width // 2
                            eng.tensor_tensor(
                                out=_ap(prod[:, 0, 0, 0:1], [[256, A], [64, E], [1, h2]]),
                                in0=_ap(prod[:, 0, 0, 0:1], [[256, A], [64, E], [1, h2]]),
                                in1=_ap(prod[:, 0, 0, h2:h2 + 1], [[256, A], [64, E], [1, h2]]),
                                op=ADD)
                            width = h2
                        # final level -> l[:, i, e, j] fp32 (iter (j, e))
                        lbase = l_t[:, i, 0, 0:1]
                        eng.tensor_tensor(
                            out=_ap(lbase, [[1, A], [A, E]]),
                            in0=_ap(prod[:, 0, 0, 0:1], [[256, A], [64, E]]),
                            in1=_ap(prod[:, 0, 0, 1:2], [[256, A], [64, E]]),
                            op=ADD)

                    # softmax over j (free dim), scale 1/sqrt(D) folded into exp
                    wexp = ap_.tile([128, A * E * A], BF16, tag="wexp")
                    nc.scalar.activation(out=wexp, in_=l_t.rearrange("p i e j -> p (i e j)"),
                                         func=EXP, scale=1.0 / np.sqrt(np.float32(D)))
                    wm = ap_.tile([128, A, E, A], BF16, tag="wm")  # (i, e, j)
                    nc.vector.tensor_tensor(out=wm.rearrange("p i e j -> p (i e j)"),
                                            in0=wexp, in1=mask_sb, op=MULT)
                    ssum = ap_.tile([128, A * E], F32, tag="ssum")
                    nc.vector.tensor_reduce(out=ssum, in_=wm.rearrange("p i e j -> p (i e) j"),
                                            axis=X, op=ADD)
                    rs = ap_.tile([128, A * E], F32, tag="rs")
                    nc.vector.reciprocal(out=rs, in_=ssum)
                    # p2[i, j, e, 2] = wm[i, e, j] * rs[i, e]  (duplicated along last dim)
                    p2 = ap_.tile([128, A, A, E, 2], BF16, tag="p2")
                    for k2 in range(2):
                        nc.vector.tensor_tensor(
                            out=_ap(p2[:, 0, 0, 0, k2:k2 + 1], [[64, A], [2, E], [8, A]]),
                            in0=_ap(wm[:, 0, 0, 0:1], [[32, A], [8, E], [1, A]]),
                            in1=_ap(rs[:, 0:1], [[4, A], [1, E], [0, A]]),
                            op=MULT)

                    # other: per i, prod2[j, e, d] = p2[i, j, e] * vals[j, e, d]; tree over j
                    for i in range(A):
                        eng = nc.vector if i < GP_SPLIT else nc.gpsimd
                        prod2 = pp.tile([128, A, E, D], BF16, tag="prod")  # (j, e, d)
                        eng.tensor_tensor(
                            out=_ap(prod2[:, 0, 0, 0:1], [[64, A * E], [2, D // 2], [1, 2]]),
                            in0=_ap(p2[:, i, 0, 0, 0:1], [[2, A * E], [0, D // 2], [1, 2]]),
                            in1=_ap(vals_bm[bt][:, 0, 0:1], [[64, A * E], [2, D // 2], [1, 2]]),
                            op=MULT)
                        jw = A
                        while jw > 2:
                            h2 = jw // 2
                            eng.tensor_tensor(
                                out=_ap(prod2[:, 0, 0, 0:1], [[1, h2 * E * D]]),
                                in0=_ap(prod2[:, 0, 0, 0:1], [[1, h2 * E * D]]),
                                in1=_ap(prod2[:, h2, 0, 0:1], [[1, h2 * E * D]]),
                                op=ADD)
                            jw = h2
                        eng.tensor_tensor(
                            out=_ap(other_bm[bt][:, i, 0:1], [[1, E * D]]),
                            in0=_ap(prod2[:, 0, 0, 0:1], [[1, E * D]]),
                            in1=_ap(prod2[:, 1, 0, 0:1], [[1, E * D]]),
                            op=ADD)

                # ---------- critic phase (per agent) ----------
                allq_bm = [op_.tile([128, A, NACT], F32, tag=f"allq{bt}", name=f"allq{bt}") for bt in range(nbt)]
                for a in range(A):
                    otherT = cp.tile([128, 2, NB], BF16, tag="otherT")
                    for kt in range(2):
                        for bt in range(nbt):
                            nc.sync.dma_start_transpose(
                                out=otherT[:, kt, bt * 128:(bt + 1) * 128],
                                in_=other_bm[bt][:, a, kt * 128:(kt + 1) * 128])
                    h_t = cp.tile([128, 2, NB], BF16, tag="h")
                    for mt in range(2):
                        ps = psp.tile([128, NB], F32, tag="mm")
                        for kt in range(4):
                            rhs = s_enc_all[:, a, kt, :] if kt < 2 else otherT[:, kt - 2, :]
                            nc.tensor.matmul(out=ps,
                                             lhsT=w_c1_sb[a][:, kt, mt * 128:(mt + 1) * 128],
                                             rhs=rhs,
                                             start=(kt == 0), stop=(kt == 3))
                        evict_lrelu(ps, h_t[:, mt, :], b_c1_sb[a][:, mt, :],
                                    b_c12_sb[a][:, mt, :] if not lrelu_act else None)
                    psq = qsp.tile([NACT, NB], F32, tag="q")
                    for kt in range(2):
                        nc.tensor.matmul(out=psq, lhsT=w_c2_sb[a][:, kt, :],
                                         rhs=h_t[:, kt, :],
                                         start=(kt == 0), stop=(kt == 1))
                    allq = cp.tile([NACT, NB], F32, tag="allq")
                    nc.scalar.activation(out=allq, in_=psq, func=IDENT,
                                         bias=b_c2_sb[a], scale=1.0)
                    for bt in range(nbt):
                        pt = tpp.tile([128, NACT], F32, tag="tp")
                        nc.tensor.transpose(out=pt, in_=allq[:, bt * 128:(bt + 1) * 128],
                                            identity=ident16)
                        nc.vector.tensor_copy(out=allq_bm[bt][:, a, :], in_=pt)

                # ---------- argmax gather (per b-tile) ----------
                for bt in range(nbt):
                    acts_t = op_.tile([128, A, NACT], F32, tag="acts")
                    nc.sync.dma_start(out=acts_t, in_=acts_d[c0 + bt * 128: c0 + (bt + 1) * 128])
                    amax = op_.tile([128, A], F32, tag="amax")
                    nc.vector.tensor_reduce(out=amax, in_=acts_t, axis=X,
                                            op=mybir.AluOpType.max)
                    onehot = op_.tile([128, A, NACT], F32, tag="onehot")
                    nc.vector.tensor_tensor(
                        out=onehot.rearrange("p a o -> p (a o)"),
                        in0=acts_t.rearrange("p a o -> p (a o)"),
                        in1=_ap(amax[:, 0:1], [[1, A], [0, NACT]]),
                        op=ISGE)
                    qm = op_.tile([128, A, NACT], F32, tag="qm")
                    nc.vector.tensor_tensor(out=qm.rearrange("p a o -> p (a o)"),
                                            in0=onehot.rearrange("p a o -> p (a o)"),
                                            in1=allq_bm[bt].rearrange("p a o -> p (a o)"),
                                            op=MULT)
                    q_sb = op_.tile([128, A], F32, tag="qsb")
                    nc.vector.tensor_reduce(out=q_sb, in_=qm, axis=X, op=ADD)
                    nc.sync.dma_start(
                        out=bass.AP(tensor=q_d, offset=c0 + bt * 128,
                                    ap=[[1, 128], [b_local, A]]),
                        in_=q_sb)

    nc.compile()
    return nc


def _prep_inputs(states, actions, enc_W, enc_b, s_W, s_b, key_W, sel_W,
                 val_W, val_b, c_W1, c_b1, c_W2, c_b2,
                 b_local=B_LOCAL, n_cores=N_CORES):
    """Host-side: build per-core input dicts (shard over B, bf16 layouts)."""
    f32 = np.float32
    B = b_local * n_cores
    states = states[:, :B]
    actions = actions[:, :B]
    # feature-major [A, KPAD, B] bf16 of [states|actions]
    sa = np.concatenate([states, actions], axis=-1).astype(f32)      # [A, B, 472]
    saT = np.zeros((A, KPAD, B), dtype=bfloat16)
    saT[:, :IDIM, :] = sa.transpose(0, 2, 1).astype(bfloat16)
    # combined encoder weights [A, KPAD, 512]: cols 0:256 enc, 256:512 s
    w_enc = np.zeros((A, KPAD, 2 * H), dtype=bfloat16)
    w_enc[:, :IDIM, :H] = enc_W.astype(bfloat16)
    w_enc[:, :S, H:] = s_W.astype(bfloat16)
    b_enc = np.concatenate([enc_b, s_b], axis=-1).astype(f32).reshape(A, 4, 128, 1)
    # kvs weights [H, 768]: keys | vals | sels, each (e,d) col order
    w_kvs = np.zeros((H, 3 * H), dtype=bfloat16)
    w_kvs[:, 0:H] = key_W.transpose(1, 0, 2).reshape(H, H).astype(bfloat16)
    w_kvs[:, H:2 * H] = val_W.transpose(1, 0, 2).reshape(H, H).astype(bfloat16)
    w_kvs[:, 2 * H:] = sel_W.transpose(1, 0, 2).reshape(H, H).astype(bfloat16)
    b_val = val_b.reshape(2, 128, 1).astype(f32)
    w_c1 = c_W1.astype(bfloat16)                                    # [A, 512, 256]
    b_c1 = c_b1.astype(f32).reshape(A, 2, 128, 1)
    w_c2 = c_W2.astype(bfloat16)                                    # [A, 256, 16]
    b_c2 = c_b2.astype(f32).reshape(A, NACT, 1)
    # mask [128, (i, e, j)]: 0 where i == j else 1
    m = np.ones((A, E, A), dtype=bfloat16)
    for i in range(A):
        m[i, :, i] = 0
    mask = np.broadcast_to(m.reshape(1, -1), (128, A * E * A)).copy()
    acts_bm = actions.transpose(1, 0, 2).astype(f32)                # [B, A, 16]

    shared = dict(w_enc=w_enc, b_enc=b_enc, w_kvs=w_kvs, b_val=b_val,
                  w_c1=w_c1, b_c1=b_c1, w_c2=w_c2, b_c2=b_c2, mask=mask)
    in_maps = []
    for cid in range(n_cores):
        sl = slice(cid * b_local, (cid + 1) * b_local)
        m_ = dict(shared)
        m_["saT"] = np.ascontiguousarray(saT[:, :, sl])
        m_["acts"] = np.ascontiguousarray(acts_bm[sl])
        in_maps.append(m_)
    return in_maps


_NC_CACHE = {}


def _get_nc(b_local=B_LOCAL, lrelu_act=True):
    key = (b_local, lrelu_act)
    if key not in _NC_CACHE:
        _NC_CACHE[key] = build_bass(b_local, lrelu_act)
    return _NC_CACHE[key]


def kernel(**inputs):
    inputs = {k: np.asarray(v) for k, v in inputs.items()}
    in_maps = _prep_inputs(**inputs)
    nc = _get_nc()
    res = bass_utils.run_bass_kernel_spmd(
        nc, in_maps, core_ids=list(range(N_CORES)),
        trace=bool(int(os.environ.get("MAAC_TRACE", "0"))))
    q = np.concatenate([r["q"] for r in res.results], axis=1)  # [A, B]
    if res.exec_time_ns is not None:
        print(f"HW exec time: {res.exec_time_ns} ns")
    return q[:, :, None].astype(np.float32)


# revision 8
# speedup vs baseline: 1.4000x; 1.4000x over previous
"""MAAC critic kernel for Trainium2, data-parallel over the batch dim on 8 cores.

Self-contained: hardcodes all shapes. The harness calls kernel(**inputs) with the
full (unsharded) inputs and gets the full [A, B, 1] output back.

Per-core dataflow (B_local=2048, chunks of 256):
  encoder (feature-major, weights-stationary matmuls, fused bias+lrelu eviction)
  -> keys/vals/sels produced batch-major (activations-stationary matmuls; vals
     bias added via a K=1 ones-row matmul) -> 8x8 agent attention as wide
     broadcast-AP tensor_tensor products + halving trees, split DVE/GPSIMD
  -> softmax over agents in the free dim (exp on ScalarE, 1/sqrt(D) in scale)
  -> attended values transposed back feature-major via bf16 DMA-transpose
  -> per-agent critic MLP -> PE-transpose of q-values -> fp32 is_ge one-hot
     gather of the taken action.
"""
import os
import numpy as np
import ml_dtypes

import concourse.bass as bass
import concourse.tile as tile
import concourse.mybir as mybir
from concourse import bacc
from concourse import bass_utils

F32 = mybir.dt.float32
BF16 = mybir.dt.bfloat16
bfloat16 = ml_dtypes.bfloat16

A = 8
B = 16384
S = 456
NACT = 16
IDIM = S + NACT          # 472
KPAD = 512               # padded encoder contraction dim
H = 256
E = 4
D = 64
ED = E * D               # 256
N_CORES = 8
B_LOCAL = B // N_CORES   # 2048
NB = 256                 # chunk size along b
ALPHA = 0.01             # leaky relu slope


def _ap(base, dims):
    """AP with base's partition dim + explicit free [step, count] dims (elements)."""
    return bass.AP(tensor=base.tensor, offset=base.offset, ap=[base.ap[0], *dims])


def _subgroups(i0, i1, max_n=3):
    out = []
    i = i0
    while i < i1:
        n = min(max_n, i1 - i)
        out.append((i, n))
        i += n
    return out


def build_bass(b_local=B_LOCAL, lrelu_act=True):
    """Build and compile the single-core Bass module (same NEFF on all cores)."""
    nc = bacc.Bacc("TRN2", target_bir_lowering=False, debug=False)
    n_chunks = b_local // NB
    LRELU = mybir.ActivationFunctionType.Lrelu
    RELU = mybir.ActivationFunctionType.Relu
    EXP = mybir.ActivationFunctionType.Exp
    IDENT = mybir.ActivationFunctionType.Identity
    MULT = mybir.AluOpType.mult
    ADD = mybir.AluOpType.add
    ISGE = mybir.AluOpType.is_ge
    MAX = mybir.AluOpType.max
    X = mybir.AxisListType.X

    # ---- DRAM I/O ----
    saT_d = nc.dram_tensor("saT", [A, KPAD, b_local], BF16, kind="ExternalInput")
    acts_d = nc.dram_tensor("acts", [b_local, A, NACT], F32, kind="ExternalInput")
    w_enc_d = nc.dram_tensor("w_enc", [A, KPAD, 2 * H], BF16, kind="ExternalInput")
    b_enc_d = nc.dram_tensor("b_enc", [A, 4, 128, 1], F32, kind="ExternalInput")
    w_kvs_d = nc.dram_tensor("w_kvs", [H, 3 * H], BF16, kind="ExternalInput")
    kvbias_d = nc.dram_tensor("kvbias", [1, 2 * H], BF16, kind="ExternalInput")
    w_c1_d = nc.dram_tensor("w_c1", [A, 2 * H, H], BF16, kind="ExternalInput")
    b_c1_d = nc.dram_tensor("b_c1", [A, 2, 128, 1], F32, kind="ExternalInput")
    w_c2_d = nc.dram_tensor("w_c2", [A, H, NACT], BF16, kind="ExternalInput")
    b_c2_d = nc.dram_tensor("b_c2", [A, NACT, 1], F32, kind="ExternalInput")
    mask_d = nc.dram_tensor("mask", [128, A * E * A], BF16, kind="ExternalInput")
    q_d = nc.dram_tensor("q", [A, b_local], F32, kind="ExternalOutput")

    with tile.TileContext(nc) as tc:
        import contextlib
        with contextlib.ExitStack() as ctx:
            wp = ctx.enter_context(tc.tile_pool(name="wp", bufs=1))
            xp = ctx.enter_context(tc.tile_pool(name="xp", bufs=3))
            ep = ctx.enter_context(tc.tile_pool(name="ep", bufs=3))
            sp = ctx.enter_context(tc.tile_pool(name="sp", bufs=2))
            bmp = ctx.enter_context(tc.tile_pool(name="bmp", bufs=2))
            pp = ctx.enter_context(tc.tile_pool(name="pp", bufs=1))
            ap_ = ctx.enter_context(tc.tile_pool(name="ap", bufs=2))
            op_ = ctx.enter_context(tc.tile_pool(name="op", bufs=2))
            cp = ctx.enter_context(tc.tile_pool(name="cp", bufs=3))
            pbig = ctx.enter_context(tc.tile_pool(name="pbig", bufs=2, space="PSUM"))
            pmed = ctx.enter_context(tc.tile_pool(name="pmed", bufs=3, space="PSUM"))
            qsp = ctx.enter_context(tc.tile_pool(name="qsp", bufs=1, space="PSUM"))
            tpp = ctx.enter_context(tc.tile_pool(name="tpp", bufs=2, space="PSUM"))

            # ---- resident weights ----
            w_enc_sb, b_enc_sb, b_enc2_sb = [], [], []
            w_c1_sb, b_c1_sb, b_c12_sb = [], [], []
            w_c2_sb, b_c2_sb = [], []
            for a in range(A):
                w = wp.tile([128, 4, 2 * H], BF16, name=f"w_enc{a}")
                nc.sync.dma_start(out=w, in_=w_enc_d[a].rearrange("(kt p) m -> p kt m", p=128))
                w_enc_sb.append(w)
                bt_ = wp.tile([128, 4, 1], F32, name=f"b_enc{a}")
                nc.sync.dma_start(out=bt_, in_=b_enc_d[a].rearrange("kt p one -> p kt one"))
                b_enc_sb.append(bt_)
                w1 = wp.tile([128, 4, H], BF16, name=f"w_c1{a}")
                nc.sync.dma_start(out=w1, in_=w_c1_d[a].rearrange("(kt p) m -> p kt m", p=128))
                w_c1_sb.append(w1)
                b1 = wp.tile([128, 2, 1], F32, name=f"b_c1{a}")
                nc.sync.dma_start(out=b1, in_=b_c1_d[a].rearrange("kt p one -> p kt one"))
                b_c1_sb.append(b1)
                w2 = wp.tile([128, 2, NACT], BF16, name=f"w_c2{a}")
                nc.sync.dma_start(out=w2, in_=w_c2_d[a].rearrange("(kt p) m -> p kt m", p=128))
                w_c2_sb.append(w2)
                b2 = wp.tile([NACT, 1], F32, name=f"b_c2{a}")
                nc.sync.dma_start(out=b2, in_=b_c2_d[a])
                b_c2_sb.append(b2)
                if not lrelu_act:
                    bt2 = wp.tile([128, 4, 1], F32, name=f"b_enc2{a}")
                    nc.vector.tensor_scalar_mul(out=bt2, in0=bt_, scalar1=-(1.0 - ALPHA))
                    b_enc2_sb.append(bt2)
                    b12 = wp.tile([128, 2, 1], F32, name=f"b_c12{a}")
                    nc.vector.tensor_scalar_mul(out=b12, in0=b1, scalar1=-(1.0 - ALPHA))
                    b_c12_sb.append(b12)
            w_kvs_sb = wp.tile([128, 2, 3 * H], BF16, name="w_kvs")
            nc.sync.dma_start(out=w_kvs_sb, in_=w_kvs_d.rearrange("(kt p) m -> p kt m", p=128))
            kvbias_sb = wp.tile([1, 2 * H], BF16, name="kvbias")
            nc.sync.dma_start(out=kvbias_sb, in_=kvbias_d[:, :])
            ones_sb = wp.tile([1, 128], BF16, name="ones")
            nc.vector.memset(ones_sb, 1.0)
            mask_sb = wp.tile([128, A * E * A], BF16, name="mask")
            nc.sync.dma_start(out=mask_sb, in_=mask_d[:, :])
            ident16 = wp.tile([16, 16], F32, name="ident16")
            from concourse.masks import make_identity
            make_identity(nc, ident16)

            def evict_lrelu(psum_ap, out_ap, bias, bias2):
                """out = lrelu(psum + bias); bias may be None (already in psum)."""
                if lrelu_act:
                    nc.scalar.activation(out=out_ap, in_=psum_ap, func=LRELU,
                                         bias=bias if bias is not None else 0.0,
                                         scale=1.0, alpha=ALPHA)
                else:
                    r = ep.tile([128, NB], F32, tag="lrtmp")
                    rr = r[:, :psum_ap.shape[-1]] if psum_ap.shape[-1] != NB else r
                    nc.scalar.activation(out=rr, in_=psum_ap, func=RELU,
                                         bias=bias2 if bias2 is not None else 0.0,
                                         scale=-(1.0 - ALPHA))
                    nc.vector.scalar_tensor_tensor(
                        out=out_ap, in0=psum_ap,
                        scalar=bias if bias is not None else 0.0,
                        op0=ADD, in1=rr, op1=ADD)

            for c in range(n_chunks):
                c0 = c * NB
                nbt = NB // 128  # b-tiles per chunk (2)

                s_enc_all = sp.tile([128, A, 2, NB], BF16, tag="s_enc")
                keys_bm = [bmp.tile([128, A, ED], BF16, tag=f"keys{bt}", name=f"keys{bt}") for bt in range(nbt)]
                vals_bm = [bmp.tile([128, A, ED], BF16, tag=f"vals{bt}", name=f"vals{bt}") for bt in range(nbt)]
                sels_bm = [bmp.tile([128, A, ED], BF16, tag=f"sels{bt}", name=f"sels{bt}") for bt in range(nbt)]

                # ---------- feature-major encoder + b-major kvs (per agent) ----------
                for a in range(A):
                    x = xp.tile([128, 4, NB], BF16, tag="x")
                    nc.sync.dma_start(
                        out=x,
                        in_=saT_d[a].rearrange("(kt p) b -> p kt b", p=128)[:, :, c0:c0 + NB])

                    sa_enc = ep.tile([128, 2, NB], BF16, tag="sa_enc")
                    for mt in range(4):
                        ps = pmed.tile([128, NB], F32, tag="mm")
                        for kt in range(4):
                            nc.tensor.matmul(out=ps,
                                             lhsT=w_enc_sb[a][:, kt, mt * 128:(mt + 1) * 128],
                                             rhs=x[:, kt, :],
                                             start=(kt == 0), stop=(kt == 3))
                        dst = sa_enc[:, mt, :] if mt < 2 else s_enc_all[:, a, mt - 2, :]
                        evict_lrelu(ps, dst, b_enc_sb[a][:, mt, :],
                                    b_enc2_sb[a][:, mt, :] if not lrelu_act else None)

                    # b-major keys|vals: psum[b, 512] = sa_enc.T @ w_kvs[:, :512] + 1^T @ kvbias
                    for bt in range(nbt):
                        ps = pbig.tile([128, 2 * H], F32, tag="kv")
                        for kt in range(2):
                            nc.tensor.matmul(out=ps,
                                             lhsT=sa_enc[:, kt, bt * 128:(bt + 1) * 128],
                                             rhs=w_kvs_sb[:, kt, 0:2 * H],
                                             start=(kt == 0), stop=False)
                        nc.tensor.matmul(out=ps, lhsT=ones_sb, rhs=kvbias_sb,
                                         start=False, stop=True)
                        nc.scalar.activation(out=keys_bm[bt][:, a, :], in_=ps[:, 0:ED],
                                             func=IDENT, bias=0.0, scale=1.0)
                        evict_lrelu(ps[:, ED:2 * ED], vals_bm[bt][:, a, :], None, None)
                        # b-major sels: psum[b, 256] = s_enc.T @ w_kvs[:, 512:]
                        ps2 = pmed.tile([128, NB], F32, tag="mm")
                        for kt in range(2):
                            nc.tensor.matmul(out=ps2[:, 0:ED],
                                             lhsT=s_enc_all[:, a, kt, bt * 128:(bt + 1) * 128],
                                             rhs=w_kvs_sb[:, kt, 2 * H:3 * H],
                                             start=(kt == 0), stop=(kt == 1))
                        nc.scalar.activation(out=sels_bm[bt][:, a, :], in_=ps2[:, 0:ED],
                                             func=IDENT, bias=0.0, scale=1.0)

                # ---------- attention (per b-tile of 128) ----------
                other_bm = [op_.tile([128, A, ED], BF16, tag=f"other{bt}", name=f"other{bt}") for bt in range(nbt)]
                for bt in range(nbt):
                    ndve = 5 - (bt & 1)  # alternate 5/3 and 4/4 DVE/GP split
                    groups = ([(nc.vector, i0, n) for i0, n in _subgroups(0, ndve)]
                              + [(nc.gpsimd, i0, n) for i0, n in _subgroups(ndve, A)])
                    l_t = ap_.tile([128, A, E, A], F32, tag="l")      # (i, e, j)
                    for eng, i0, ni in groups:
                        tagp = "prodv" if eng is nc.vector else "prodg"
                        prod = pp.tile([128, 3, A, ED], BF16, tag=tagp, name=tagp)
                        # prod[i, j, (e d)] = sels[i, (e d)] * keys[j, (e d)]
                        if eng is nc.vector:
                            # contiguous per-(i,j) ops hit the 2x DVE mode
                            for ii in range(ni):
                                for j in range(A):
                                    eng.tensor_tensor(
                                        out=prod[:, ii, j, :],
                                        in0=sels_bm[bt][:, i0 + ii, :],
                                        in1=keys_bm[bt][:, j, :],
                                        op=MULT)
                        else:
                            eng.tensor_tensor(
                                out=_ap(prod[:, 0, 0, 0:1], [[A * ED, ni], [ED, A], [1, ED]]),
                                in0=_ap(sels_bm[bt][:, i0, 0:1], [[ED, ni], [0, A], [1, ED]]),
                                in1=_ap(keys_bm[bt][:, 0, 0:1], [[0, ni], [ED, A], [1, ED]]),
                                op=MULT)
                        # d-tree: sum over d (innermost), (i, j, e) kept
                        width = D
                        while width > 2:
                            h2 = width // 2
                            eng.tensor_tensor(
                                out=_ap(prod[:, 0, 0, 0:1], [[ED, ni * A], [D, E], [1, h2]]),
                                in0=_ap(prod[:, 0, 0, 0:1], [[ED, ni * A], [D, E], [1, h2]]),
                                in1=_ap(prod[:, 0, 0, h2:h2 + 1], [[ED, ni * A], [D, E], [1, h2]]),
                                op=ADD)
                            width = h2
                        # final level -> l[:, i0:i0+ni, e, j] fp32 (iter (i, j, e))
                        eng.tensor_tensor(
                            out=_ap(l_t[:, i0, 0, 0:1], [[E * A, ni], [1, A], [A, E]]),
                            in0=_ap(prod[:, 0, 0, 0:1], [[A * ED, ni], [ED, A], [D, E]]),
                            in1=_ap(prod[:, 0, 0, 1:2], [[A * ED, ni], [ED, A], [D, E]]),
                            op=ADD)

                    # softmax over j; 1/sqrt(D) folded into exp scale
                    wexp = ap_.tile([128, A * E * A], BF16, tag="wexp")
                    nc.scalar.activation(out=wexp, in_=l_t.rearrange("p i e j -> p (i e j)"),
                                         func=EXP, scale=1.0 / np.sqrt(np.float32(D)))
                    wm = ap_.tile([128, A, E, A], BF16, tag="wm")  # (i, e, j)
                    nc.vector.tensor_tensor(out=wm.rearrange("p i e j -> p (i e j)"),
                                            in0=wexp, in1=mask_sb, op=MULT)
                    ssum = ap_.tile([128, A * E], F32, tag="ssum")
                    nc.vector.tensor_reduce(out=ssum, in_=wm.rearrange("p i e j -> p (i e) j"),
                                            axis=X, op=ADD)
                    rs = ap_.tile([128, A * E], F32, tag="rs")
                    nc.vector.reciprocal(out=rs, in_=ssum)
                    # p2[i, j, e] = wm[i, e, j] * rs[i, e]
                    p2 = ap_.tile([128, A, A, E], BF16, tag="p2")
                    nc.vector.tensor_tensor(
                        out=_ap(p2[:, 0, 0, 0:1], [[A * E, A], [1, E], [E, A]]),
                        in0=_ap(wm[:, 0, 0, 0:1], [[E * A, A], [A, E], [1, A]]),
                        in1=_ap(rs[:, 0:1], [[E, A], [1, E], [0, A]]),
                        op=MULT)

                    # other[i, (e d)] = sum_j p2[i, j, e] * vals[j, (e d)]
                    for eng, i0, ni in groups:
                        tagp = "prodv" if eng is nc.vector else "prodg"
                        prod2 = pp.tile([128, 3, A, ED], BF16, tag=tagp, name=tagp)
                        eng.tensor_tensor(
                            out=_ap(prod2[:, 0, 0, 0:1],
                                    [[A * ED, ni], [D, A * E], [1, D]]),
                            in0=_ap(p2[:, i0, 0, 0:1],
                                    [[A * E, ni], [1, A * E], [0, D]]),
                            in1=_ap(vals_bm[bt][:, 0, 0:1],
                                    [[0, ni], [D, A * E], [1, D]]),
                            op=MULT)
                        jw = A
                        while jw > 2:
                            h2 = jw // 2
                            eng.tensor_tensor(
                                out=_ap(prod2[:, 0, 0, 0:1], [[A * ED, ni], [1, h2 * ED]]),
                                in0=_ap(prod2[:, 0, 0, 0:1], [[A * ED, ni], [1, h2 * ED]]),
                                in1=_ap(prod2[:, 0, h2, 0:1], [[A * ED, ni], [1, h2 * ED]]),
                                op=ADD)
                            jw = h2
                        eng.tensor_tensor(
                            out=_ap(other_bm[bt][:, i0, 0:1], [[ED, ni], [1, ED]]),
                            in0=_ap(prod2[:, 0, 0, 0:1], [[A * ED, ni], [1, ED]]),
                            in1=_ap(prod2[:, 0, 1, 0:1], [[A * ED, ni], [1, ED]]),
                            op=ADD)

                # ---------- critic (per agent) ----------
                allq_bm = [op_.tile([128, A, NACT], F32, tag=f"allq{bt}", name=f"allq{bt}") for bt in range(nbt)]
                for a in range(A):
                    otherT = cp.tile([128, 2, NB], BF16, tag="otherT")
                    for kt in range(2):
                        for bt in range(nbt):
                            nc.sync.dma_start_transpose(
                                out=otherT[:, kt, bt * 128:(bt + 1) * 128],
                                in_=other_bm[bt][:, a, kt * 128:(kt + 1) * 128])
                    h_t = cp.tile([128, 2, NB], BF16, tag="h")
                    for mt in range(2):
                        ps = pmed.tile([128, NB], F32, tag="mm")
                        for kt in range(4):
                            rhs = s_enc_all[:, a, kt, :] if kt < 2 else otherT[:, kt - 2, :]
                            nc.tensor.matmul(out=ps,
                                             lhsT=w_c1_sb[a][:, kt, mt * 128:(mt + 1) * 128],
                                             rhs=rhs,
                                             start=(kt == 0), stop=(kt == 3))
                        evict_lrelu(ps, h_t[:, mt, :], b_c1_sb[a][:, mt, :],
                                    b_c12_sb[a][:, mt, :] if not lrelu_act else None)
                    psq = qsp.tile([NACT, NB], F32, tag="q")
                    for kt in range(2):
                        nc.tensor.matmul(out=psq, lhsT=w_c2_sb[a][:, kt, :],
                                         rhs=h_t[:, kt, :],
                                         start=(kt == 0), stop=(kt == 1))
                    allq = cp.tile([NACT, NB], F32, tag="allq")
                    nc.scalar.activation(out=allq, in_=psq, func=IDENT,
                                         bias=b_c2_sb[a], scale=1.0)
                    for bt in range(nbt):
                        pt = tpp.tile([128, NACT], F32, tag="tp")
                        nc.tensor.transpose(out=pt, in_=allq[:, bt * 128:(bt + 1) * 128],
                                            identity=ident16)
                        nc.vector.tensor_copy(out=allq_bm[bt][:, a, :], in_=pt)

                # ---------- argmax gather (per b-tile) ----------
                for bt in range(nbt):
                    acts_t = op_.tile([128, A, NACT], F32, tag="acts")
                    nc.sync.dma_start(out=acts_t, in_=acts_d[c0 + bt * 128: c0 + (bt + 1) * 128])
                    amax = op_.tile([128, A], F32, tag="amax")
                    nc.vector.tensor_reduce(out=amax, in_=acts_t, axis=X, op=MAX)
                    onehot = op_.tile([128, A, NACT], F32, tag="onehot")
                    nc.vector.tensor_tensor(
                        out=onehot.rearrange("p a o -> p (a o)"),
                        in0=acts_t.rearrange("p a o -> p (a o)"),
                        in1=_ap(amax[:, 0:1], [[1, A], [0, NACT]]),
                        op=ISGE)
                    qm = op_.tile([128, A, NACT], F32, tag="qm")
                    nc.vector.tensor_tensor(out=qm.rearrange("p a o -> p (a o)"),
                                            in0=onehot.rearrange("p a o -> p (a o)"),
                                            in1=allq_bm[bt].rearrange("p a o -> p (a o)"),
                                            op=MULT)
                    q_sb = op_.tile([128, A], F32, tag="qsb")
                    nc.vector.tensor_reduce(out=q_sb, in_=qm, axis=X, op=ADD)
                    nc.sync.dma_start(
                        out=bass.AP(tensor=q_d, offset=c0 + bt * 128,
                                    ap=[[1, 128], [b_local, A]]),
                        in_=q_sb)

    nc.compile()
    return nc


def _prep_inputs(states, actions, enc_W, enc_b, s_W, s_b, key_W, sel_W,
                 val_W, val_b, c_W1, c_b1, c_W2, c_b2,
                 b_local=B_LOCAL, n_cores=N_CORES):
    """Host-side: build per-core input dicts (shard over B, bf16 layouts)."""
    f32 = np.float32
    Bv = b_local * n_cores
    states = states[:, :Bv]
    actions = actions[:, :Bv]
    sa = np.concatenate([states, actions], axis=-1).astype(f32)      # [A, Bv, 472]
    saT = np.zeros((A, KPAD, Bv), dtype=bfloat16)
    saT[:, :IDIM, :] = sa.transpose(0, 2, 1).astype(bfloat16)
    w_enc = np.zeros((A, KPAD, 2 * H), dtype=bfloat16)
    w_enc[:, :IDIM, :H] = enc_W.astype(bfloat16)
    w_enc[:, :S, H:] = s_W.astype(bfloat16)
    b_enc = np.concatenate([enc_b, s_b], axis=-1).astype(f32).reshape(A, 4, 128, 1)
    w_kvs = np.zeros((H, 3 * H), dtype=bfloat16)
    w_kvs[:, 0:H] = key_W.transpose(1, 0, 2).reshape(H, H).astype(bfloat16)
    w_kvs[:, H:2 * H] = val_W.transpose(1, 0, 2).reshape(H, H).astype(bfloat16)
    w_kvs[:, 2 * H:] = sel_W.transpose(1, 0, 2).reshape(H, H).astype(bfloat16)
    kvbias = np.zeros((1, 2 * H), dtype=bfloat16)
    kvbias[0, H:] = val_b.reshape(-1).astype(bfloat16)
    w_c1 = c_W1.astype(bfloat16)
    b_c1 = c_b1.astype(f32).reshape(A, 2, 128, 1)
    w_c2 = c_W2.astype(bfloat16)
    b_c2 = c_b2.astype(f32).reshape(A, NACT, 1)
    m = np.ones((A, E, A), dtype=bfloat16)
    for i in range(A):
        m[i, :, i] = 0
    mask = np.broadcast_to(m.reshape(1, -1), (128, A * E * A)).copy()
    acts_bm = actions.transpose(1, 0, 2).astype(f32)                # [Bv, A, 16]

    shared = dict(w_enc=w_enc, b_enc=b_enc, w_kvs=w_kvs, kvbias=kvbias,
                  w_c1=w_c1, b_c1=b_c1, w_c2=w_c2, b_c2=b_c2, mask=mask)
    in_maps = []
    for cid in range(n_cores):
        sl = slice(cid * b_local, (cid + 1) * b_local)
        m_ = dict(shared)
        m_["saT"] = np.ascontiguousarray(saT[:, :, sl])
        m_["acts"] = np.ascontiguousarray(acts_bm[sl])
        in_maps.append(m_)
    return in_maps


_NC_CACHE = {}


def _get_nc(b_local=B_LOCAL, lrelu_act=True):
    key = (b_local, lrelu_act)
    if key not in _NC_CACHE:
        _NC_CACHE[key] = build_bass(b_local, lrelu_act)
    return _NC_CACHE[key]


def kernel(**inputs):
    inputs = {k: np.asarray(v) for k, v in inputs.items()}
    in_maps = _prep_inputs(**inputs)
    nc = _get_nc()
    res = bass_utils.run_bass_kernel_spmd(
        nc, in_maps, core_ids=list(range(N_CORES)),
        trace=bool(int(os.environ.get("MAAC_TRACE", "0"))))
    q = np.concatenate([r["q"] for r in res.results], axis=1)  # [A, B]
    if res.exec_time_ns is not None:
        print(f"HW exec time: {res.exec_time_ns} ns")
    return q[:, :, None].astype(np.float32)
